# revision 25
# baseline (speedup 1.0000x reference)
"""Fused RoPE attention + LayerNorm, Trainium2, 8 NeuronCores (SPMD).

Head-parallel sharding: core c owns head pair (2c, 2c+1) and computes
Q/K/V projections + attention for the FULL sequence (both batches) for
its two heads.  Inputs x_qk / x_v are replicated to every core (DMA,
not collectives), so the big K/V AllGathers of the row-sharded design
disappear.  The only collective is a tiny LayerNorm-stats AllReduce
(each core holds 128 of the 1024 columns of attn output; row mean/var
need all 1024), overlapped with attention of the other batch.

RoPE is applied without cross-partition shuffles: rot2(U) is a signed
pair-permutation, computed on the TensorEngine as perm^T @ U with a
constant [128,128] matrix; then q_rot = U*cos + rot2(U)*sin (DVE+Pool).

Engine split: PE matmuls; Act psum->sbuf proj copies + exp + LN sqrt;
DVE RoPE muls/adds, aT casts, recip, stats, LN math+normalize;
Pool (gpsimd) RoPE sin-mul, V pack copies, attn normalize.
"""
import sys
import types
import os
import numpy as np
from contextlib import ExitStack

for _p in ("/opt/trn_rl_repo",):
    if _p not in sys.path:
        sys.path.append(_p)

# NTFF profile hook shim: lets BASS_TRACE=1 work in images whose antenv
# lacks axon_hooks (bass_utils imports it when tracing under axon).
if "antenv.axon_hooks" not in sys.modules:
    _hooks = types.ModuleType("antenv.axon_hooks")
    _HOOK = [None]
    _hooks.set_axon_ntff_profile_hook = lambda h: _HOOK.__setitem__(0, h)
    _hooks.get_axon_ntff_profile_hook = lambda: _HOOK[0]
    sys.modules["antenv.axon_hooks"] = _hooks
    try:
        from trn_agent_boot.trn_boot import _ntff_profile_via_ctypes

        _HOOK[0] = _ntff_profile_via_ctypes("/opt/axon/libaxon_pjrt.so")
    except Exception:
        pass

import concourse.bass as bass  # noqa: E402
import concourse.bacc as bacc  # noqa: E402
import concourse.mybir as mybir  # noqa: E402
import concourse.tile as tile  # noqa: E402
from concourse import bass_utils  # noqa: E402

F32 = mybir.dt.float32
BF16 = mybir.dt.bfloat16
NP_BF16 = np.dtype(mybir.dt.np(BF16))
AF = mybir.ActivationFunctionType
ALU = mybir.AluOpType
AX = mybir.AxisListType

B, S, D, H, DH = 2, 2048, 1024, 16, 64
NC = 8
R = B * S             # 4096 rows (positions across both batches)
DC = D // 128         # 8 contraction chunks
NSEG = 8              # projection segments of 512 positions
SEGW = R // NSEG      # 512
KT = 32               # global 128-key tiles (16 per batch)
NQB = 4               # 512-wide q blocks per batch
LN_EPS = 1e-5
ROPE_BASE = 10000.0


def _build(flags):
    has_bqk, has_bv, has_gb = flags
    nc = bacc.Bacc("TRN2", target_bir_lowering=False, debug=False,
                   num_devices=NC)

    xqT = nc.dram_tensor("xqT", [D, R], BF16, kind="ExternalInput")
    xvT = nc.dram_tensor("xvT", [D, R], BF16, kind="ExternalInput")
    wq_d = nc.dram_tensor("wq", [D, 128], BF16, kind="ExternalInput")
    wk_d = nc.dram_tensor("wk", [D, 128], BF16, kind="ExternalInput")
    wv_d = nc.dram_tensor("wv", [D, 128], BF16, kind="ExternalInput")
    perm_d = nc.dram_tensor("perm", [128, 128], BF16, kind="ExternalInput")
    ident_d = nc.dram_tensor("ident", [128, 128], BF16, kind="ExternalInput")
    cos_d = nc.dram_tensor("cos", [128, R], F32, kind="ExternalInput")
    sin_d = nc.dram_tensor("sin", [128, R], F32, kind="ExternalInput")
    if has_bqk:
        cq_d = nc.dram_tensor("cq", [128, R], F32, kind="ExternalInput")
        ck_d = nc.dram_tensor("ck", [128, R], F32, kind="ExternalInput")
    if has_bv:
        bv_d = nc.dram_tensor("bv", [128, 128], F32, kind="ExternalInput")
    if has_gb:
        gam_d = nc.dram_tensor("gamma", [128, 128], F32, kind="ExternalInput")
        bet_d = nc.dram_tensor("beta", [128, 128], F32, kind="ExternalInput")
    out_d = nc.dram_tensor("out", [R, 128], F32, kind="ExternalOutput")

    es = ExitStack()
    with es:
        tc = es.enter_context(tile.TileContext(nc))
        dram = es.enter_context(
            tc.tile_pool(name="dram", bufs=1, space="DRAM"))
        constp = es.enter_context(tc.tile_pool(name="const", bufs=1))
        qkp = es.enter_context(tc.tile_pool(name="qkp", bufs=1))
        vfp = es.enter_context(tc.tile_pool(name="vfp", bufs=1))
        ptp = es.enter_context(tc.tile_pool(name="ptp", bufs=2))
        attnp = es.enter_context(tc.tile_pool(name="attnp", bufs=1))
        statp = es.enter_context(tc.tile_pool(name="statp", bufs=1))
        lnp = es.enter_context(tc.tile_pool(name="lnp", bufs=2))
        outp = es.enter_context(tc.tile_pool(name="outp", bufs=4))

        st_b = [dram.tile([128, 32], F32, tag=f"stb{b}", name=f"stb{b}")
                for b in range(B)]
        st_g = [dram.tile([NC * 128, 32], F32, tag=f"stg{b}", name=f"stg{b}",
                          addr_space="Shared")
                for b in range(B)]

        cos_sb = constp.tile([128, R], F32, tag="cos")
        sin_sb = constp.tile([128, R], F32, tag="sin")
        perm_sb = constp.tile([128, 128], BF16, tag="perm")
        ident_sb = constp.tile([128, 128], BF16, tag="ident")
        eps_sb = constp.tile([128, 1], F32, tag="eps")
        nc.vector.memset(eps_sb[:], LN_EPS)

        cq_sb = ck_sb = bv_sb = gam_sb = bet_sb = None
        if has_bqk:
            cq_sb = constp.tile([128, R], F32, tag="cq")
            ck_sb = constp.tile([128, R], F32, tag="ck")
            for hf in range(2):
                sl = slice(hf * 2048, (hf + 1) * 2048)
                nc.sync.dma_start(cq_sb[:, sl], cq_d[:, sl])
                nc.sync.dma_start(ck_sb[:, sl], ck_d[:, sl])
        if has_bv:
            bv_sb = constp.tile([128, 128], F32, tag="bvs")
            nc.sync.dma_start(bv_sb[:], bv_d[:])
        if has_gb:
            gam_sb = constp.tile([128, 128], F32, tag="gam")
            nc.sync.dma_start(gam_sb[:], gam_d[:])
            bet_sb = constp.tile([128, 128], F32, tag="bet")
            nc.sync.dma_start(bet_sb[:], bet_d[:])

        # [dh-of-pair (h0: 0-63, h1: 64-127), b*2048 + s]
        q_sb = qkp.tile([128, R], BF16, tag="q")
        k_sb = qkp.tile([128, R], BF16, tag="k")
        q1_sb = qkp.tile([64, R], BF16, tag="q1")   # partitions 64:128 -> 0:64
        k1_sb = qkp.tile([64, R], BF16, tag="k1")
        # [key-in-tile, kt*130 + hl*65 + (dh | ones)]
        v_sb = vfp.tile([128, KT * 130], BF16, tag="v")
        # [q-in-tile, tt*128 + hl*64 + dh] for row-tile tt
        attn_sb = attnp.tile([128, 32 * 128], F32, tag="attn")
        # [row, tt*2 + (sum|sumsq)]
        stats_sb = statp.tile([128, 64], F32, tag="stats")

        # ---------------- projections ----------------
        pes = ExitStack()
        with pes:
            xqp = pes.enter_context(tc.tile_pool(name="xqp", bufs=3))
            xvp = pes.enter_context(tc.tile_pool(name="xvp", bufs=2))
            wp = pes.enter_context(tc.tile_pool(name="wp", bufs=1))
            usbp = pes.enter_context(tc.tile_pool(name="usbp", bufs=4))
            stage = pes.enter_context(tc.tile_pool(name="stage", bufs=6))
            pjp = pes.enter_context(
                tc.tile_pool(name="pjp", bufs=6, space="PSUM"))
            pvp = pes.enter_context(
                tc.tile_pool(name="pvp", bufs=2, space="PSUM"))

            def load_w(t_dram, tg):
                # one 3-D DMA: [dc, p, col] -> [p, dc*128 + col]
                w_sb = wp.tile([128, DC * 128], BF16, tag=tg)
                nc.sync.dma_start(
                    w_sb[:].rearrange("p (dc c) -> p dc c", c=128),
                    t_dram[:].rearrange("(dc p) c -> p dc c", p=128))
                return w_sb

            wq_sb = load_w(wq_d, "wq")
            wk_sb = load_w(wk_d, "wk")
            wv_sb = load_w(wv_d, "wv")
            nc.sync.dma_start(perm_sb[:], perm_d[:])
            nc.sync.dma_start(ident_sb[:], ident_d[:])

            def load_xseg(pool, src, seg, tg):
                # one 3-D DMA per segment: [dc, p, col] -> [p, dc*SEGW + col]
                t = pool.tile([128, DC * SEGW], BF16, tag=tg)
                nc.sync.dma_start(
                    t[:].rearrange("p (dc c) -> p dc c", c=SEGW),
                    src[:, seg * SEGW:(seg + 1) * SEGW].rearrange(
                        "(dc p) c -> p dc c", p=128))
                return t

            # ones in v_sb (cols kt*130 + hl*65 + 64)
            v3 = v_sb[:].rearrange("p (x e) -> p x e", e=65)
            nc.gpsimd.memset(v3[:, :, 64:65], 1.0)

            def proj_u(w_sb, xseg, seg, nm):
                ps_u = pjp.tile([128, SEGW], F32, tag="pj",
                                name=f"psu_{nm}_{seg}")
                for dc in range(DC):
                    nc.tensor.matmul(
                        ps_u[:],
                        w_sb[:, dc * 128:(dc + 1) * 128],
                        xseg[:, dc * SEGW:(dc + 1) * SEGW],
                        start=(dc == 0), stop=(dc == DC - 1))
                return ps_u

            def rope(ps_u, seg, c_sb, dst, dst1, nm):
                sl = slice(seg * SEGW, (seg + 1) * SEGW)
                u_sb = usbp.tile([128, SEGW], BF16, tag="usb",
                                 name=f"usb_{nm}_{seg}")
                nc.scalar.copy(u_sb[:], ps_u[:])                     # Act
                ps_u2 = pjp.tile([128, SEGW], F32, tag="pj",
                                 name=f"psu2_{nm}_{seg}")
                nc.tensor.matmul(ps_u2[:], perm_sb[:], u_sb[:],
                                 start=True, stop=True)
                t1 = stage.tile([128, SEGW], F32, tag="st",
                                name=f"t1_{nm}_{seg}")
                nc.vector.tensor_mul(t1[:], ps_u[:], cos_sb[:, sl])  # DVE
                t2 = stage.tile([128, SEGW], F32, tag="st",
                                name=f"t2_{nm}_{seg}")
                nc.vector.tensor_mul(t2[:], ps_u2[:], sin_sb[:, sl])  # DVE
                if c_sb is None:
                    nc.vector.tensor_add(dst[:, sl], t1[:], t2[:])   # DVE
                else:
                    t3 = stage.tile([128, SEGW], F32, tag="st",
                                    name=f"t3_{nm}_{seg}")
                    nc.vector.tensor_add(t3[:], t1[:], t2[:])
                    nc.vector.tensor_add(dst[:, sl], t3[:], c_sb[:, sl])
                nc.sync.dma_start(dst1[:, sl], dst[64:128, sl])

            def v_proj(xvseg, seg):
                for j in range(4):
                    kt = seg * 4 + j
                    ps_v = pvp.tile([128, 128], F32, tag="pv",
                                    name=f"psv_{kt}")
                    for dc in range(DC):
                        nc.tensor.matmul(
                            ps_v[:],
                            xvseg[:, dc * SEGW + j * 128:
                                  dc * SEGW + (j + 1) * 128],
                            wv_sb[:, dc * 128:(dc + 1) * 128],
                            start=(dc == 0), stop=(dc == DC - 1))
                    dstv = v_sb[:, kt * 130:(kt + 1) * 130].rearrange(
                        "p (h e) -> p h e", e=65)[:, :, 0:64]
                    srcv = ps_v[:].rearrange("p (h d) -> p h d", d=64)
                    if has_bv:
                        bvv = bv_sb[:].rearrange("p (h d) -> p h d", d=64)
                        nc.vector.tensor_add(dstv, srcv, bvv)
                    else:
                        nc.vector.tensor_copy(dstv, srcv)            # DVE

            for hf in range(4):
                sl = slice(hf * 1024, (hf + 1) * 1024)
                nc.sync.dma_start(cos_sb[:, sl], cos_d[:, sl])
                nc.sync.dma_start(sin_sb[:, sl], sin_d[:, sl])
            for seg in range(NSEG):
                xseg = load_xseg(xqp, xqT, seg, "xq")
                ps_q = proj_u(wq_sb, xseg, seg, "q")
                ps_k = proj_u(wk_sb, xseg, seg, "k")
                rope(ps_q, seg, cq_sb, q_sb, q1_sb, "q")
                rope(ps_k, seg, ck_sb, k_sb, k1_sb, "k")
            for seg in range(NSEG):
                xvseg = load_xseg(xvp, xvT, seg, "xv")
                v_proj(xvseg, seg)

        # Full cross-engine fence: attention-phase PSUM tiles reuse the
        # projection pools' banks, and a matmul start=True zeroes its whole
        # 2KB PSUM bank -- without the fence that clobbers proj psum tiles
        # that still have pending readers (zero-region isn't visible to the
        # tile dependency tracker).
        with tc.tile_critical(name="proj_done"):
            pass

        # ---------------- attention ----------------
        aes = ExitStack()
        with aes:
            scp = aes.enter_context(
                tc.tile_pool(name="scp", bufs=2, space="PSUM"))
            avp = aes.enter_context(
                tc.tile_pool(name="avp", bufs=2, space="PSUM"))
            trp = aes.enter_context(
                tc.tile_pool(name="trp", bufs=2, space="PSUM"))
            atsb = aes.enter_context(tc.tile_pool(name="atsb", bufs=2))
            recp = aes.enter_context(tc.tile_pool(name="recp", bufs=2))
            sqp = aes.enter_context(tc.tile_pool(name="sqp", bufs=2))

            # unit = (b, qb, hl); per batch, qb-major with hl inner so a
            # (qb, both heads) pair completes 4 row-tiles of attn_sb.
            units = [(b, qb, hl)
                     for b in range(B) for qb in range(NQB) for hl in range(2)]

            def stage1(u):
                """scores + exp -> pt tile (returned)."""
                b, qb, hl = u
                ksrc = k_sb if hl == 0 else k1_sb
                qsrc = q_sb if hl == 0 else q1_sb
                qsl = slice(b * S + qb * 512, b * S + (qb + 1) * 512)
                pt = ptp.tile([128, 16 * 512], BF16, tag="pt",
                              name=f"pt_{b}_{qb}_{hl}")
                for st in range(8):
                    ps_s = scp.tile([128, 1024], F32, tag="sc",
                                    name=f"sc_{b}_{qb}_{hl}_{st}")
                    for j in range(2):
                        ktb = st * 2 + j
                        nc.tensor.matmul(
                            ps_s[:, j * 512:(j + 1) * 512],
                            ksrc[0:64, b * S + ktb * 128:
                                 b * S + (ktb + 1) * 128],
                            qsrc[0:64, qsl],
                            start=True, stop=True)
                    nc.scalar.activation(
                        pt[:, st * 1024:(st + 1) * 1024], ps_s[:],
                        AF.Exp, scale=0.125)
                return pt

            def stage2(u, pt):
                """AV + transpose + normalize -> attn_sb columns."""
                b, qb, hl = u
                aT = avp.tile([65, 512], F32, tag="av",
                              name=f"aT_{b}_{qb}_{hl}")
                for ktb in range(16):
                    kt = b * 16 + ktb
                    nc.tensor.matmul(
                        aT[:],
                        v_sb[:, kt * 130 + hl * 65: kt * 130 + hl * 65 + 65],
                        pt[:, ktb * 512:(ktb + 1) * 512],
                        start=(ktb == 0), stop=(ktb == 15))
                aT_sb = atsb.tile([65, 512], BF16, tag="ats",
                                  name=f"ats_{b}_{qb}_{hl}")
                nc.vector.tensor_copy(aT_sb[:], aT[:])               # DVE
                tr = trp.tile([128, 264], BF16, tag="tr",
                              name=f"tr_{b}_{qb}_{hl}")
                for t in range(4):
                    nc.tensor.transpose(
                        tr[:, t * 66: t * 66 + 65],
                        aT_sb[:, t * 128:(t + 1) * 128],
                        ident_sb[0:65, 0:65])
                tr_sb = atsb.tile([128, 260], BF16, tag="trs",
                                  name=f"trs_{b}_{qb}_{hl}")
                nc.vector.tensor_copy(                               # DVE 2x
                    tr_sb[:].rearrange("p (t e) -> p t e", e=65),
                    tr[:].rearrange("p (t e) -> p t e", e=66)[:, :, 0:65])
                rec = recp.tile([128, 4], F32, tag="rec",
                                name=f"rec_{b}_{qb}_{hl}")
                nc.vector.reciprocal(rec[:], tr_sb[:, 64::65])       # DVE
                for t in range(4):
                    tt = b * 16 + qb * 4 + t
                    nc.vector.tensor_scalar(                         # DVE
                        attn_sb[:, tt * 128 + hl * 64:
                                tt * 128 + hl * 64 + 64],
                        tr_sb[:, t * 65: t * 65 + 64],
                        rec[:, t: t + 1], None, ALU.mult)

            def stats_qb(b, qb):
                for t in range(4):
                    tt = b * 16 + qb * 4 + t
                    at = attn_sb[:, tt * 128:(tt + 1) * 128]
                    nc.vector.reduce_sum(
                        stats_sb[:, 2 * tt: 2 * tt + 1], at, axis=AX.X)
                    sq = sqp.tile([128, 128], F32, tag="sq",
                                  name=f"sq_{tt}")
                    nc.vector.tensor_mul(sq[:], at, at)
                    nc.vector.reduce_sum(
                        stats_sb[:, 2 * tt + 1: 2 * tt + 2], sq[:],
                        axis=AX.X)

            def stats_flush(b):
                nc.sync.dma_start(st_b[b][:],
                                  stats_sb[:, b * 32:(b + 1) * 32])
                nc.gpsimd.collective_compute(
                    "AllGather", ALU.bypass,
                    ins=[st_b[b][:].opt()], outs=[st_g[b][:].opt()],
                    replica_groups=[list(range(NC))])

            def ln_half(b, lnp, outp):
                tot8 = lnp.tile([128, 8 * 32], F32, tag="tot8",
                                name=f"tot8{b}")
                for c in range(NC):
                    nc.sync.dma_start(tot8[:, c * 32:(c + 1) * 32],
                                      st_g[b][c * 128:(c + 1) * 128, :])
                tot = lnp.tile([128, 32], F32, tag="tot", name=f"tot{b}")
                nc.vector.tensor_add(tot[:], tot8[:, 0:32], tot8[:, 32:64])
                for c in range(2, NC):
                    nc.vector.tensor_add(tot[:], tot[:],
                                         tot8[:, c * 32:(c + 1) * 32])
                nmu = lnp.tile([128, 16], F32, tag="nmu", name=f"nmu{b}")
                nc.vector.tensor_scalar_mul(nmu[:], tot[:, 0::2], -1.0 / D)
                ex2 = lnp.tile([128, 16], F32, tag="ex2", name=f"ex2{b}")
                nc.vector.tensor_scalar_mul(ex2[:], tot[:, 1::2], 1.0 / D)
                var = lnp.tile([128, 16], F32, tag="var", name=f"var{b}")
                nc.vector.tensor_tensor(var[:], nmu[:], nmu[:], ALU.mult)
                nc.vector.tensor_tensor(var[:], ex2[:], var[:], ALU.subtract)
                std = lnp.tile([128, 16], F32, tag="std", name=f"std{b}")
                nc.scalar.activation(std[:], var[:], AF.Sqrt,
                                     bias=eps_sb[:])                 # Act
                rstd = lnp.tile([128, 16], F32, tag="rstd", name=f"rstd{b}")
                nc.vector.reciprocal(rstd[:], std[:])
                mrs = lnp.tile([128, 16], F32, tag="mrs", name=f"mrs{b}")
                nc.vector.tensor_tensor(mrs[:], nmu[:], rstd[:], ALU.mult)
                for t in range(16):
                    tt = b * 16 + t
                    o_sb = outp.tile([128, 128], F32, tag="o",
                                     name=f"o_{tt}")
                    nc.vector.tensor_scalar(                         # DVE
                        o_sb[:], attn_sb[:, tt * 128:(tt + 1) * 128],
                        rstd[:, t: t + 1], mrs[:, t: t + 1],
                        ALU.mult, ALU.add)
                    if has_gb:
                        nc.vector.tensor_tensor(
                            o_sb[:], o_sb[:], gam_sb[:], ALU.mult)
                        nc.vector.tensor_tensor(
                            o_sb[:], o_sb[:], bet_sb[:], ALU.add)
                    nc.sync.dma_start(out_d[tt * 128:(tt + 1) * 128, :],
                                      o_sb[:])

            # KSTAGE: 2 = attention only; 25 = +stats; 26 = +collective
            # +readback; 30 = full (LN math + normalize)
            KSTAGE = int(os.environ.get("KSTAGE", "30"))
            pend = None
            for i, u in enumerate(units):
                pt = stage1(u)
                if pend is not None:
                    stage2(*pend)
                    pb, pqb, phl = pend[0]
                    if KSTAGE >= 25 and phl == 1:
                        stats_qb(pb, pqb)
                        if KSTAGE >= 26 and pqb == NQB - 1:
                            stats_flush(pb)
                pend = (u, pt)
            stage2(*pend)
            if KSTAGE >= 25:
                stats_qb(B - 1, NQB - 1)
            if KSTAGE >= 26:
                stats_flush(B - 1)
            if KSTAGE >= 30:
                # The LN tail must not be scheduled into the attention
                # stream: its first ops wait on the stats AllGathers, and
                # hoisting them blocks the whole in-order DVE queue (observed
                # 50us engine stalls).  tile_critical + a nested TileContext
                # (the supported pattern) pins them after the attention
                # drain, with their own pools.
                with tc.tile_critical(name="ln_tail"):
                    with tile.TileContext(nc) as tc2:
                        with tc2.tile_pool(name="lnp2", bufs=2) as lnp2, \
                                tc2.tile_pool(name="outp2", bufs=4) as outp2:
                            ln_half(0, lnp2, outp2)
                            ln_half(1, lnp2, outp2)
            else:
                if KSTAGE >= 26:
                    # read back the gathered stats (sums unused downstream)
                    for b in range(B):
                        tot8 = lnp.tile([128, 8 * 32], F32, tag="tot8",
                                        name=f"tot8d{b}")
                        for c in range(NC):
                            nc.sync.dma_start(
                                tot8[:, c * 32:(c + 1) * 32],
                                st_g[b][c * 128:(c + 1) * 128, :])
                        tot = lnp.tile([128, 32], F32, tag="tot",
                                       name=f"totd{b}")
                        nc.vector.tensor_add(tot[:], tot8[:, 0:32],
                                             tot8[:, 32:64])
                # debug: dump raw attn (no LN) so outputs are produced
                for tt in range(32):
                    o_sb = outp.tile([128, 128], F32, tag="o",
                                     name=f"od_{tt}")
                    nc.vector.tensor_copy(
                        o_sb[:], attn_sb[:, tt * 128:(tt + 1) * 128])
                    nc.sync.dma_start(out_d[tt * 128:(tt + 1) * 128, :],
                                      o_sb[:])

    nc.compile()
    return nc


_CACHE: dict = {}
LAST_EXEC_NS = None


def _rope_tables():
    half = DH // 2
    inv_freq = 1.0 / (ROPE_BASE ** (np.arange(half, dtype=np.float32) / half))
    t = np.arange(S, dtype=np.float32)
    freqs = t[:, None] * inv_freq[None, :]
    emb = np.concatenate([freqs, freqs], axis=-1)          # [S, DH]
    return np.cos(emb).astype(np.float32), np.sin(emb).astype(np.float32)


def prep_flags(inputs):
    b_qk = np.asarray(inputs["b_qk"], dtype=np.float32)
    b_v = np.asarray(inputs["b_v"], dtype=np.float32)
    gamma = np.asarray(inputs["ln_gamma"], dtype=np.float32)
    beta = np.asarray(inputs["ln_beta"], dtype=np.float32)
    return (bool(np.any(b_qk)), bool(np.any(b_v)),
            bool(np.any(gamma != 1.0) or np.any(beta != 0.0)))


def _perm_mat():
    Pm = np.zeros((128, 128), np.float32)
    for i in range(64):
        Pm[2 * i + 1, 2 * i] = -1.0
        Pm[2 * i, 2 * i + 1] = 1.0
    return Pm


def _prep_in_maps(inputs, flags):
    x_qk = np.asarray(inputs["x_qk"], dtype=np.float32)
    x_v = np.asarray(inputs["x_v"], dtype=np.float32)
    W_qk = np.asarray(inputs["W_qk"], dtype=np.float32)
    b_qk = np.asarray(inputs["b_qk"], dtype=np.float32)
    W_v = np.asarray(inputs["W_v"], dtype=np.float32)
    b_v = np.asarray(inputs["b_v"], dtype=np.float32)
    gamma = np.asarray(inputs["ln_gamma"], dtype=np.float32)
    beta = np.asarray(inputs["ln_beta"], dtype=np.float32)

    Pm = _perm_mat()
    Pm64 = Pm[:DH, :DH]
    cos_all, sin_all = _rope_tables()          # [S, 64]
    cos_in = np.ascontiguousarray(np.tile(cos_all.T, (2, 2)))  # [128, 4096]
    sin_in = np.ascontiguousarray(np.tile(sin_all.T, (2, 2)))

    Wq = W_qk[:, :D]
    Wk = W_qk[:, D:]
    bq = b_qk[:D].reshape(H, DH)
    bk = b_qk[D:].reshape(H, DH)
    bq2 = bq @ Pm64
    bk2 = bk @ Pm64

    xqT_np = np.ascontiguousarray(
        x_qk.reshape(R, D).T.astype(NP_BF16))
    xvT_np = np.ascontiguousarray(
        x_v.reshape(R, D).T.astype(NP_BF16))
    perm_np = np.ascontiguousarray(Pm.astype(NP_BF16))
    ident_np = np.ascontiguousarray(np.eye(128, dtype=NP_BF16))

    in_maps = []
    for c in range(NC):
        cols = slice(c * 128, (c + 1) * 128)
        m = {
            "xqT": xqT_np, "xvT": xvT_np,
            "wq": np.ascontiguousarray(Wq[:, cols].astype(NP_BF16)),
            "wk": np.ascontiguousarray(Wk[:, cols].astype(NP_BF16)),
            "wv": np.ascontiguousarray(W_v[:, cols].astype(NP_BF16)),
            "perm": perm_np, "ident": ident_np,
            "cos": cos_in, "sin": sin_in,
        }
        if flags[0]:
            # additive post-RoPE bias tables for this head pair
            def fold(bh, bh2):
                rows = [bh[2 * c + hl][:, None] * cos_all.T
                        + bh2[2 * c + hl][:, None] * sin_all.T
                        for hl in range(2)]          # each [64, S]
                return np.ascontiguousarray(
                    np.tile(np.vstack(rows), (1, 2)).astype(np.float32))
            m["cq"] = fold(bq, bq2)
            m["ck"] = fold(bk, bk2)
        if flags[1]:
            m["bv"] = np.ascontiguousarray(np.broadcast_to(
                b_v[c * 128:(c + 1) * 128], (128, 128)).astype(np.float32))
        if flags[2]:
            m["gamma"] = np.ascontiguousarray(np.broadcast_to(
                gamma[c * 128:(c + 1) * 128], (128, 128)).astype(np.float32))
            m["beta"] = np.ascontiguousarray(np.broadcast_to(
                beta[c * 128:(c + 1) * 128], (128, 128)).astype(np.float32))
        in_maps.append(m)
    return in_maps


def kernel(**inputs):
    flags = prep_flags(inputs)
    if flags not in _CACHE:
        _CACHE[flags] = _build(flags)
    nc = _CACHE[flags]
    in_maps = _prep_in_maps(inputs, flags)
    res = bass_utils.run_bass_kernel_spmd(
        nc, in_maps, core_ids=list(range(NC)))
    global LAST_EXEC_NS
    LAST_EXEC_NS = res.exec_time_ns
    out = np.empty((R, D), np.float32)
    for c in range(NC):
        out[:, c * 128:(c + 1) * 128] = np.asarray(
            res.results[c]["out"], dtype=np.float32)
    return out.reshape(B, S, D)


# revision 32
# speedup vs baseline: 1.0933x; 1.0933x over previous
"""Fused RoPE attention + LayerNorm, Trainium2, 8 NeuronCores (SPMD).

Head-parallel sharding: core c owns head pair (2c, 2c+1) and computes
Q/K/V projections + attention for the FULL sequence (both batches) for
its two heads.  Inputs x_qk / x_v are replicated to every core (DMA,
not collectives), so the big K/V AllGathers of the row-sharded design
disappear.  The only collective is a tiny LayerNorm-stats AllReduce
(each core holds 128 of the 1024 columns of attn output; row mean/var
need all 1024), overlapped with attention of the other batch.

RoPE is applied without cross-partition shuffles: rot2(U) is a signed
pair-permutation, computed on the TensorEngine as perm^T @ U with a
constant [128,128] matrix; then q_rot = U*cos + rot2(U)*sin (DVE+Pool).

Engine split: PE matmuls; Act psum->sbuf proj copies + exp + LN sqrt;
DVE RoPE muls/adds, aT casts, recip, stats, LN math+normalize;
Pool (gpsimd) RoPE sin-mul, V pack copies, attn normalize.
"""
import sys
import types
import os
import numpy as np
from contextlib import ExitStack

for _p in ("/opt/trn_rl_repo",):
    if _p not in sys.path:
        sys.path.append(_p)

# NTFF profile hook shim: lets BASS_TRACE=1 work in images whose antenv
# lacks axon_hooks (bass_utils imports it when tracing under axon).
if "antenv.axon_hooks" not in sys.modules:
    _hooks = types.ModuleType("antenv.axon_hooks")
    _HOOK = [None]
    _hooks.set_axon_ntff_profile_hook = lambda h: _HOOK.__setitem__(0, h)
    _hooks.get_axon_ntff_profile_hook = lambda: _HOOK[0]
    sys.modules["antenv.axon_hooks"] = _hooks
    try:
        from trn_agent_boot.trn_boot import _ntff_profile_via_ctypes

        _HOOK[0] = _ntff_profile_via_ctypes("/opt/axon/libaxon_pjrt.so")
    except Exception:
        pass

import concourse.bass as bass  # noqa: E402
import concourse.bacc as bacc  # noqa: E402
import concourse.mybir as mybir  # noqa: E402
import concourse.tile as tile  # noqa: E402
from concourse import bass_utils  # noqa: E402

F32 = mybir.dt.float32
BF16 = mybir.dt.bfloat16
NP_BF16 = np.dtype(mybir.dt.np(BF16))
AF = mybir.ActivationFunctionType
ALU = mybir.AluOpType
AX = mybir.AxisListType

B, S, D, H, DH = 2, 2048, 1024, 16, 64
NC = 8
R = B * S             # 4096 rows (positions across both batches)
DC = D // 128         # 8 contraction chunks
NSEG = 8              # projection segments of 512 positions
SEGW = R // NSEG      # 512
KT = 32               # global 128-key tiles (16 per batch)
NQB = 4               # 512-wide q blocks per batch
LN_EPS = 1e-5
ROPE_BASE = 10000.0


def _build(flags):
    has_bqk, has_bv, has_gb = flags
    nc = bacc.Bacc("TRN2", target_bir_lowering=False, debug=False,
                   num_devices=NC)

    xqT = nc.dram_tensor("xqT", [NSEG * 128, DC * SEGW], BF16,
                         kind="ExternalInput")
    xvT = nc.dram_tensor("xvT", [NSEG * 128, DC * SEGW], BF16,
                         kind="ExternalInput")
    wq_d = nc.dram_tensor("wq", [D, 128], BF16, kind="ExternalInput")
    wk_d = nc.dram_tensor("wk", [D, 128], BF16, kind="ExternalInput")
    wv_d = nc.dram_tensor("wv", [D, 128], BF16, kind="ExternalInput")
    perm_d = nc.dram_tensor("perm", [128, 128], BF16, kind="ExternalInput")
    ident_d = nc.dram_tensor("ident", [128, 128], BF16, kind="ExternalInput")
    cos_d = nc.dram_tensor("cos", [128, R], F32, kind="ExternalInput")
    sin_d = nc.dram_tensor("sin", [128, R], F32, kind="ExternalInput")
    if has_bqk:
        cq_d = nc.dram_tensor("cq", [128, R], F32, kind="ExternalInput")
        ck_d = nc.dram_tensor("ck", [128, R], F32, kind="ExternalInput")
    if has_bv:
        bv_d = nc.dram_tensor("bv", [128, 128], F32, kind="ExternalInput")
    if has_gb:
        gam_d = nc.dram_tensor("gamma", [128, 128], F32, kind="ExternalInput")
        bet_d = nc.dram_tensor("beta", [128, 128], F32, kind="ExternalInput")
    out_d = nc.dram_tensor("out", [R, 128], F32, kind="ExternalOutput")

    es = ExitStack()
    with es:
        tc = es.enter_context(tile.TileContext(nc))
        dram = es.enter_context(
            tc.tile_pool(name="dram", bufs=1, space="DRAM"))
        constp = es.enter_context(tc.tile_pool(name="const", bufs=1))
        qkp = es.enter_context(tc.tile_pool(name="qkp", bufs=1))
        vfp = es.enter_context(tc.tile_pool(name="vfp", bufs=1))
        ptp = es.enter_context(tc.tile_pool(name="ptp", bufs=2))
        attnp = es.enter_context(tc.tile_pool(name="attnp", bufs=1))
        statp = es.enter_context(tc.tile_pool(name="statp", bufs=1))
        lnp = es.enter_context(tc.tile_pool(name="lnp", bufs=2))
        outp = es.enter_context(tc.tile_pool(name="outp", bufs=4))

        st_b = [dram.tile([128, 32], F32, tag=f"stb{b}", name=f"stb{b}")
                for b in range(B)]
        st_g = [dram.tile([NC * 128, 32], F32, tag=f"stg{b}", name=f"stg{b}",
                          addr_space="Shared")
                for b in range(B)]

        cos_sb = constp.tile([128, R], F32, tag="cos")
        sin_sb = constp.tile([128, R], F32, tag="sin")
        perm_sb = constp.tile([128, 128], BF16, tag="perm")
        ident_sb = constp.tile([128, 128], BF16, tag="ident")
        eps_sb = constp.tile([128, 1], F32, tag="eps")
        nc.vector.memset(eps_sb[:], LN_EPS)

        cq_sb = ck_sb = bv_sb = gam_sb = bet_sb = None
        if has_bqk:
            cq_sb = constp.tile([128, R], F32, tag="cq")
            ck_sb = constp.tile([128, R], F32, tag="ck")
            for hf in range(2):
                sl = slice(hf * 2048, (hf + 1) * 2048)
                nc.sync.dma_start(cq_sb[:, sl], cq_d[:, sl])
                nc.sync.dma_start(ck_sb[:, sl], ck_d[:, sl])
        if has_bv:
            bv_sb = constp.tile([128, 128], F32, tag="bvs")
            nc.sync.dma_start(bv_sb[:], bv_d[:])
        if has_gb:
            gam_sb = constp.tile([128, 128], F32, tag="gam")
            nc.sync.dma_start(gam_sb[:], gam_d[:])
            bet_sb = constp.tile([128, 128], F32, tag="bet")
            nc.sync.dma_start(bet_sb[:], bet_d[:])

        # [dh-of-pair (h0: 0-63, h1: 64-127), b*2048 + s]
        q_sb = qkp.tile([128, R], BF16, tag="q")
        k_sb = qkp.tile([128, R], BF16, tag="k")
        q1_sb = qkp.tile([64, R], BF16, tag="q1")   # partitions 64:128 -> 0:64
        k1_sb = qkp.tile([64, R], BF16, tag="k1")
        # [key-in-tile, kt*130 + hl*65 + (dh | ones)]
        v_sb = vfp.tile([128, KT * 130], BF16, tag="v")
        # [q-in-tile, tt*128 + hl*64 + dh] for row-tile tt
        attn_sb = attnp.tile([128, 32 * 128], F32, tag="attn")
        # [row, tt*2 + (sum|sumsq)]
        stats_sb = statp.tile([128, 64], F32, tag="stats")

        # ---------------- projections ----------------
        pes = ExitStack()
        with pes:
            xqp = pes.enter_context(tc.tile_pool(name="xqp", bufs=3))
            xvp = pes.enter_context(tc.tile_pool(name="xvp", bufs=2))
            wp = pes.enter_context(tc.tile_pool(name="wp", bufs=1))
            usbp = pes.enter_context(tc.tile_pool(name="usbp", bufs=4))
            stage = pes.enter_context(tc.tile_pool(name="stage", bufs=6))
            pjp = pes.enter_context(
                tc.tile_pool(name="pjp", bufs=6, space="PSUM"))
            pvp = pes.enter_context(
                tc.tile_pool(name="pvp", bufs=2, space="PSUM"))

            def load_w(t_dram, tg):
                # one 3-D DMA: [dc, p, col] -> [p, dc*128 + col]
                w_sb = wp.tile([128, DC * 128], BF16, tag=tg)
                nc.sync.dma_start(
                    w_sb[:].rearrange("p (dc c) -> p dc c", c=128),
                    t_dram[:].rearrange("(dc p) c -> p dc c", p=128))
                return w_sb

            def load_xseg(pool, src, seg, tg, eng):
                # host pre-tiled: one plain 2-D DMA per segment
                t = pool.tile([128, DC * SEGW], BF16, tag=tg)
                eng.dma_start(t[:], src[seg * 128:(seg + 1) * 128, :])
                return t

            wq_sb = load_w(wq_d, "wq")
            xsegs = [load_xseg(xqp, xqT, s_, "xq", nc.sync)
                     for s_ in range(3)]
            wk_sb = load_w(wk_d, "wk")
            wv_sb = load_w(wv_d, "wv")
            nc.gpsimd.dma_start(perm_sb[:], perm_d[:])
            nc.gpsimd.dma_start(ident_sb[:], ident_d[:])
            for hf in range(4):
                slh = slice(hf * 1024, (hf + 1) * 1024)
                nc.gpsimd.dma_start(cos_sb[:, slh], cos_d[:, slh])
                nc.gpsimd.dma_start(sin_sb[:, slh], sin_d[:, slh])

            # ones in v_sb (cols kt*130 + hl*65 + 64)
            v3 = v_sb[:].rearrange("p (x e) -> p x e", e=65)
            nc.gpsimd.memset(v3[:, :, 64:65], 1.0)

            def proj_u(w_sb, xseg, seg, nm):
                ps_u = pjp.tile([128, SEGW], F32, tag="pj",
                                name=f"psu_{nm}_{seg}")
                for dc in range(DC):
                    nc.tensor.matmul(
                        ps_u[:],
                        w_sb[:, dc * 128:(dc + 1) * 128],
                        xseg[:, dc * SEGW:(dc + 1) * SEGW],
                        start=(dc == 0), stop=(dc == DC - 1))
                return ps_u

            def rope(ps_u, seg, c_sb, dst, dst1, nm):
                sl = slice(seg * SEGW, (seg + 1) * SEGW)
                u_sb = usbp.tile([128, SEGW], BF16, tag="usb",
                                 name=f"usb_{nm}_{seg}")
                nc.scalar.copy(u_sb[:], ps_u[:])                     # Act
                ps_u2 = pjp.tile([128, SEGW], F32, tag="pj",
                                 name=f"psu2_{nm}_{seg}")
                nc.tensor.matmul(ps_u2[:], perm_sb[:], u_sb[:],
                                 start=True, stop=True)
                t1 = stage.tile([128, SEGW], F32, tag="st",
                                name=f"t1_{nm}_{seg}")
                nc.vector.tensor_mul(t1[:], ps_u[:], cos_sb[:, sl])  # DVE
                t2 = stage.tile([128, SEGW], F32, tag="st",
                                name=f"t2_{nm}_{seg}")
                nc.vector.tensor_mul(t2[:], ps_u2[:], sin_sb[:, sl])  # DVE
                if c_sb is None:
                    nc.vector.tensor_add(dst[:, sl], t1[:], t2[:])   # DVE
                else:
                    t3 = stage.tile([128, SEGW], F32, tag="st",
                                    name=f"t3_{nm}_{seg}")
                    nc.vector.tensor_add(t3[:], t1[:], t2[:])
                    nc.vector.tensor_add(dst[:, sl], t3[:], c_sb[:, sl])
                nc.scalar.dma_start(dst1[:, sl], dst[64:128, sl])

            def v_proj(xvseg, seg):
                for j in range(4):
                    kt = seg * 4 + j
                    ps_v = pvp.tile([128, 128], F32, tag="pv",
                                    name=f"psv_{kt}")
                    for dc in range(DC):
                        nc.tensor.matmul(
                            ps_v[:],
                            xvseg[:, dc * SEGW + j * 128:
                                  dc * SEGW + (j + 1) * 128],
                            wv_sb[:, dc * 128:(dc + 1) * 128],
                            start=(dc == 0), stop=(dc == DC - 1))
                    dstv = v_sb[:, kt * 130:(kt + 1) * 130].rearrange(
                        "p (h e) -> p h e", e=65)[:, :, 0:64]
                    srcv = ps_v[:].rearrange("p (h d) -> p h d", d=64)
                    if has_bv:
                        bvv = bv_sb[:].rearrange("p (h d) -> p h d", d=64)
                        nc.vector.tensor_add(dstv, srcv, bvv)
                    else:
                        nc.vector.tensor_copy(dstv, srcv)            # DVE

            for seg in range(NSEG):
                xseg = xsegs[seg] if seg < 3 else load_xseg(
                    xqp, xqT, seg, "xq", nc.sync)
                ps_q = proj_u(wq_sb, xseg, seg, "q")
                ps_k = proj_u(wk_sb, xseg, seg, "k")
                rope(ps_q, seg, cq_sb, q_sb, q1_sb, "q")
                rope(ps_k, seg, ck_sb, k_sb, k1_sb, "k")
            for seg in range(NSEG):
                xvseg = load_xseg(xvp, xvT, seg, "xv", nc.gpsimd)
                v_proj(xvseg, seg)

        # Full cross-engine fence: attention-phase PSUM tiles reuse the
        # projection pools' banks, and a matmul start=True zeroes its whole
        # 2KB PSUM bank -- without the fence that clobbers proj psum tiles
        # that still have pending readers (zero-region isn't visible to the
        # tile dependency tracker).
        with tc.tile_critical(name="proj_done"):
            pass

        # ---------------- attention ----------------
        aes = ExitStack()
        with aes:
            scp = aes.enter_context(
                tc.tile_pool(name="scp", bufs=2, space="PSUM"))
            avp = aes.enter_context(
                tc.tile_pool(name="avp", bufs=1, space="PSUM"))
            trp = aes.enter_context(
                tc.tile_pool(name="trp", bufs=1, space="PSUM"))
            atsb = aes.enter_context(tc.tile_pool(name="atsb", bufs=2))
            recp = aes.enter_context(tc.tile_pool(name="recp", bufs=2))
            sqp = aes.enter_context(tc.tile_pool(name="sqp", bufs=2))

            # unit = (b, qb, hl); per batch, qb-major with hl inner so a
            # (qb, both heads) pair completes 4 row-tiles of attn_sb.
            units = [(b, qb, hl)
                     for b in range(B) for qb in range(NQB) for hl in range(2)]

            def stage1(u):
                """scores + exp -> pt tile (returned)."""
                b, qb, hl = u
                ksrc = k_sb if hl == 0 else k1_sb
                qsrc = q_sb if hl == 0 else q1_sb
                qsl = slice(b * S + qb * 512, b * S + (qb + 1) * 512)
                pt = ptp.tile([128, 16 * 512], BF16, tag="pt",
                              name=f"pt_{b}_{qb}_{hl}")
                # kt chunks of (3,3,3,3,2,2): bigger ACTIVATE instrs
                # amortize the ~480ns fixed overhead on the exp stream.
                kt0 = 0
                for st, w in enumerate((3, 3, 3, 3, 2, 2)):
                    ps_s = scp.tile([128, 1536], F32, tag="sc",
                                    name=f"sc_{b}_{qb}_{hl}_{st}")
                    for j in range(w):
                        ktb = kt0 + j
                        nc.tensor.matmul(
                            ps_s[:, j * 512:(j + 1) * 512],
                            ksrc[0:64, b * S + ktb * 128:
                                 b * S + (ktb + 1) * 128],
                            qsrc[0:64, qsl],
                            start=True, stop=True)
                    nc.scalar.activation(
                        pt[:, kt0 * 512:(kt0 + w) * 512],
                        ps_s[:, 0:w * 512], AF.Exp, scale=0.125)
                    kt0 += w
                return pt

            def stage2(u, pt):
                """AV + transpose + normalize -> attn_sb columns."""
                b, qb, hl = u
                aT = avp.tile([65, 512], F32, tag="av",
                              name=f"aT_{b}_{qb}_{hl}")
                for ktb in range(16):
                    kt = b * 16 + ktb
                    nc.tensor.matmul(
                        aT[:],
                        v_sb[:, kt * 130 + hl * 65: kt * 130 + hl * 65 + 65],
                        pt[:, ktb * 512:(ktb + 1) * 512],
                        start=(ktb == 0), stop=(ktb == 15))
                aT_sb = atsb.tile([65, 512], BF16, tag="ats",
                                  name=f"ats_{b}_{qb}_{hl}")
                nc.vector.tensor_copy(aT_sb[:], aT[:])               # DVE
                tr = trp.tile([128, 264], BF16, tag="tr",
                              name=f"tr_{b}_{qb}_{hl}")
                for t in range(4):
                    nc.tensor.transpose(
                        tr[:, t * 66: t * 66 + 65],
                        aT_sb[:, t * 128:(t + 1) * 128],
                        ident_sb[0:65, 0:65])
                tr_sb = atsb.tile([128, 260], BF16, tag="trs",
                                  name=f"trs_{b}_{qb}_{hl}")
                nc.vector.tensor_copy(                               # DVE 2x
                    tr_sb[:].rearrange("p (t e) -> p t e", e=65),
                    tr[:].rearrange("p (t e) -> p t e", e=66)[:, :, 0:65])
                rec = recp.tile([128, 4], F32, tag="rec",
                                name=f"rec_{b}_{qb}_{hl}")
                nc.vector.reciprocal(rec[:], tr_sb[:, 64::65])       # DVE
                for t in range(4):
                    tt = b * 16 + qb * 4 + t
                    nc.vector.tensor_scalar(                         # DVE
                        attn_sb[:, tt * 128 + hl * 64:
                                tt * 128 + hl * 64 + 64],
                        tr_sb[:, t * 65: t * 65 + 64],
                        rec[:, t: t + 1], None, ALU.mult)

            def stats_qb(b, qb):
                for t in range(4):
                    tt = b * 16 + qb * 4 + t
                    at = attn_sb[:, tt * 128:(tt + 1) * 128]
                    nc.vector.reduce_sum(
                        stats_sb[:, 2 * tt: 2 * tt + 1], at, axis=AX.X)
                    sq = sqp.tile([128, 128], F32, tag="sq",
                                  name=f"sq_{tt}")
                    nc.vector.tensor_mul(sq[:], at, at)
                    nc.vector.reduce_sum(
                        stats_sb[:, 2 * tt + 1: 2 * tt + 2], sq[:],
                        axis=AX.X)

            def stats_flush(b):
                nc.sync.dma_start(st_b[b][:],
                                  stats_sb[:, b * 32:(b + 1) * 32])
                nc.gpsimd.collective_compute(
                    "AllGather", ALU.bypass,
                    ins=[st_b[b][:].opt()], outs=[st_g[b][:].opt()],
                    replica_groups=[list(range(NC))])

            def ln_half(b, lnp, outp):
                tot8 = lnp.tile([128, 8 * 32], F32, tag="tot8",
                                name=f"tot8{b}")
                nc.sync.dma_start(
                    tot8[:].rearrange("p (c w) -> p c w", w=32),
                    st_g[b][:].rearrange("(c p) w -> p c w", p=128))
                tot = lnp.tile([128, 32], F32, tag="tot", name=f"tot{b}")
                nc.vector.tensor_add(tot[:], tot8[:, 0:32], tot8[:, 32:64])
                for c in range(2, NC):
                    nc.vector.tensor_add(tot[:], tot[:],
                                         tot8[:, c * 32:(c + 1) * 32])
                nmu = lnp.tile([128, 16], F32, tag="nmu", name=f"nmu{b}")
                nc.vector.tensor_scalar_mul(nmu[:], tot[:, 0::2], -1.0 / D)
                ex2 = lnp.tile([128, 16], F32, tag="ex2", name=f"ex2{b}")
                nc.vector.tensor_scalar_mul(ex2[:], tot[:, 1::2], 1.0 / D)
                var = lnp.tile([128, 16], F32, tag="var", name=f"var{b}")
                nc.vector.tensor_tensor(var[:], nmu[:], nmu[:], ALU.mult)
                nc.vector.tensor_tensor(var[:], ex2[:], var[:], ALU.subtract)
                std = lnp.tile([128, 16], F32, tag="std", name=f"std{b}")
                nc.scalar.activation(std[:], var[:], AF.Sqrt,
                                     bias=eps_sb[:])                 # Act
                rstd = lnp.tile([128, 16], F32, tag="rstd", name=f"rstd{b}")
                nc.vector.reciprocal(rstd[:], std[:])
                mrs = lnp.tile([128, 16], F32, tag="mrs", name=f"mrs{b}")
                nc.vector.tensor_tensor(mrs[:], nmu[:], rstd[:], ALU.mult)
                o_sb = outp.tile([128, 16 * 128], F32, tag="o",
                                 name=f"o_{b}")
                for t in range(16):
                    tt = b * 16 + t
                    osl = o_sb[:, t * 128:(t + 1) * 128]
                    nc.vector.tensor_scalar(                         # DVE
                        osl, attn_sb[:, tt * 128:(tt + 1) * 128],
                        rstd[:, t: t + 1], mrs[:, t: t + 1],
                        ALU.mult, ALU.add)
                    if has_gb:
                        nc.vector.tensor_tensor(
                            osl, osl, gam_sb[:], ALU.mult)
                        nc.vector.tensor_tensor(
                            osl, osl, bet_sb[:], ALU.add)
                # one 3-D DMA: [p, t, col] -> out rows (b*16+t)*128+p
                nc.sync.dma_start(
                    out_d[b * 2048:(b + 1) * 2048, :].rearrange(
                        "(t p) c -> p t c", p=128),
                    o_sb[:].rearrange("p (t c) -> p t c", c=128))

            # KSTAGE: 2 = attention only; 25 = +stats; 26 = +collective
            # +readback; 30 = full (LN math + normalize)
            KSTAGE = int(os.environ.get("KSTAGE", "30"))
            pend = None
            for i, u in enumerate(units):
                pt = stage1(u)
                if pend is not None:
                    stage2(*pend)
                    pb, pqb, phl = pend[0]
                    if KSTAGE >= 25 and phl == 1:
                        stats_qb(pb, pqb)
                        if KSTAGE >= 26 and pqb == NQB - 1:
                            stats_flush(pb)
                pend = (u, pt)
            stage2(*pend)
            if KSTAGE >= 25:
                stats_qb(B - 1, NQB - 1)
            if KSTAGE >= 26:
                stats_flush(B - 1)
            if KSTAGE >= 30:
                # The LN tail must not be scheduled into the attention
                # stream: its first ops wait on the stats AllGathers, and
                # hoisting them blocks the whole in-order DVE queue (observed
                # 50us engine stalls).  tile_critical + a nested TileContext
                # (the supported pattern) pins them after the attention
                # drain, with their own pools.
                with tc.tile_critical(name="ln_tail"):
                    with tile.TileContext(nc) as tc2:
                        with tc2.tile_pool(name="lnp2", bufs=2) as lnp2, \
                                tc2.tile_pool(name="outp2", bufs=2) as outp2:
                            ln_half(0, lnp2, outp2)
                            ln_half(1, lnp2, outp2)
            else:
                if KSTAGE >= 26:
                    # read back the gathered stats (sums unused downstream)
                    for b in range(B):
                        tot8 = lnp.tile([128, 8 * 32], F32, tag="tot8",
                                        name=f"tot8d{b}")
                        for c in range(NC):
                            nc.sync.dma_start(
                                tot8[:, c * 32:(c + 1) * 32],
                                st_g[b][c * 128:(c + 1) * 128, :])
                        tot = lnp.tile([128, 32], F32, tag="tot",
                                       name=f"totd{b}")
                        nc.vector.tensor_add(tot[:], tot8[:, 0:32],
                                             tot8[:, 32:64])
                # debug: dump raw attn (no LN) so outputs are produced
                for tt in range(32):
                    o_sb = outp.tile([128, 128], F32, tag="o",
                                     name=f"od_{tt}")
                    nc.vector.tensor_copy(
                        o_sb[:], attn_sb[:, tt * 128:(tt + 1) * 128])
                    nc.sync.dma_start(out_d[tt * 128:(tt + 1) * 128, :],
                                      o_sb[:])

    nc.compile()
    return nc


_CACHE: dict = {}
LAST_EXEC_NS = None


def _rope_tables():
    half = DH // 2
    inv_freq = 1.0 / (ROPE_BASE ** (np.arange(half, dtype=np.float32) / half))
    t = np.arange(S, dtype=np.float32)
    freqs = t[:, None] * inv_freq[None, :]
    emb = np.concatenate([freqs, freqs], axis=-1)          # [S, DH]
    return np.cos(emb).astype(np.float32), np.sin(emb).astype(np.float32)


def prep_flags(inputs):
    b_qk = np.asarray(inputs["b_qk"], dtype=np.float32)
    b_v = np.asarray(inputs["b_v"], dtype=np.float32)
    gamma = np.asarray(inputs["ln_gamma"], dtype=np.float32)
    beta = np.asarray(inputs["ln_beta"], dtype=np.float32)
    return (bool(np.any(b_qk)), bool(np.any(b_v)),
            bool(np.any(gamma != 1.0) or np.any(beta != 0.0)))


def _perm_mat():
    Pm = np.zeros((128, 128), np.float32)
    for i in range(64):
        Pm[2 * i + 1, 2 * i] = -1.0
        Pm[2 * i, 2 * i + 1] = 1.0
    return Pm


def _prep_in_maps(inputs, flags):
    x_qk = np.asarray(inputs["x_qk"], dtype=np.float32)
    x_v = np.asarray(inputs["x_v"], dtype=np.float32)
    W_qk = np.asarray(inputs["W_qk"], dtype=np.float32)
    b_qk = np.asarray(inputs["b_qk"], dtype=np.float32)
    W_v = np.asarray(inputs["W_v"], dtype=np.float32)
    b_v = np.asarray(inputs["b_v"], dtype=np.float32)
    gamma = np.asarray(inputs["ln_gamma"], dtype=np.float32)
    beta = np.asarray(inputs["ln_beta"], dtype=np.float32)

    Pm = _perm_mat()
    Pm64 = Pm[:DH, :DH]
    cos_all, sin_all = _rope_tables()          # [S, 64]
    cos_in = np.ascontiguousarray(np.tile(cos_all.T, (2, 2)))  # [128, 4096]
    sin_in = np.ascontiguousarray(np.tile(sin_all.T, (2, 2)))

    Wq = W_qk[:, :D]
    Wk = W_qk[:, D:]
    bq = b_qk[:D].reshape(H, DH)
    bk = b_qk[D:].reshape(H, DH)
    bq2 = bq @ Pm64
    bk2 = bk @ Pm64

    def seg_tile(x):
        # [seg*128 + p, dc*SEGW + c] = x[seg*SEGW + c, dc*128 + p]
        return np.ascontiguousarray(
            x.reshape(NSEG, SEGW, DC, 128).transpose(0, 3, 2, 1)
            .reshape(NSEG * 128, DC * SEGW).astype(NP_BF16))

    xqT_np = seg_tile(x_qk.reshape(R, D))
    xvT_np = seg_tile(x_v.reshape(R, D))
    perm_np = np.ascontiguousarray(Pm.astype(NP_BF16))
    ident_np = np.ascontiguousarray(np.eye(128, dtype=NP_BF16))

    in_maps = []
    for c in range(NC):
        cols = slice(c * 128, (c + 1) * 128)
        m = {
            "xqT": xqT_np, "xvT": xvT_np,
            "wq": np.ascontiguousarray(Wq[:, cols].astype(NP_BF16)),
            "wk": np.ascontiguousarray(Wk[:, cols].astype(NP_BF16)),
            "wv": np.ascontiguousarray(W_v[:, cols].astype(NP_BF16)),
            "perm": perm_np, "ident": ident_np,
            "cos": cos_in, "sin": sin_in,
        }
        if flags[0]:
            # additive post-RoPE bias tables for this head pair
            def fold(bh, bh2):
                rows = [bh[2 * c + hl][:, None] * cos_all.T
                        + bh2[2 * c + hl][:, None] * sin_all.T
                        for hl in range(2)]          # each [64, S]
                return np.ascontiguousarray(
                    np.tile(np.vstack(rows), (1, 2)).astype(np.float32))
            m["cq"] = fold(bq, bq2)
            m["ck"] = fold(bk, bk2)
        if flags[1]:
            m["bv"] = np.ascontiguousarray(np.broadcast_to(
                b_v[c * 128:(c + 1) * 128], (128, 128)).astype(np.float32))
        if flags[2]:
            m["gamma"] = np.ascontiguousarray(np.broadcast_to(
                gamma[c * 128:(c + 1) * 128], (128, 128)).astype(np.float32))
            m["beta"] = np.ascontiguousarray(np.broadcast_to(
                beta[c * 128:(c + 1) * 128], (128, 128)).astype(np.float32))
        in_maps.append(m)
    return in_maps


def kernel(**inputs):
    flags = prep_flags(inputs)
    if flags not in _CACHE:
        _CACHE[flags] = _build(flags)
    nc = _CACHE[flags]
    in_maps = _prep_in_maps(inputs, flags)
    res = bass_utils.run_bass_kernel_spmd(
        nc, in_maps, core_ids=list(range(NC)))
    global LAST_EXEC_NS
    LAST_EXEC_NS = res.exec_time_ns
    out = np.empty((R, D), np.float32)
    for c in range(NC):
        out[:, c * 128:(c + 1) * 128] = np.asarray(
            res.results[c]["out"], dtype=np.float32)
    return out.reshape(B, S, D)


# revision 37
# speedup vs baseline: 1.1743x; 1.0742x over previous
"""Fused RoPE attention + LayerNorm, Trainium2, 8 NeuronCores (SPMD).

Head-parallel sharding: core c owns head pair (2c, 2c+1) and computes
Q/K/V projections + attention for the FULL sequence (both batches) for
its two heads.  Inputs x_qk / x_v are replicated to every core (DMA,
not collectives), so the big K/V AllGathers of the row-sharded design
disappear.  The only collective is a tiny LayerNorm-stats AllReduce
(each core holds 128 of the 1024 columns of attn output; row mean/var
need all 1024), overlapped with attention of the other batch.

RoPE is applied without cross-partition shuffles: rot2(U) is a signed
pair-permutation, computed on the TensorEngine as perm^T @ U with a
constant [128,128] matrix; then q_rot = U*cos + rot2(U)*sin (DVE+Pool).

Engine split: PE matmuls; Act psum->sbuf proj copies + exp + LN sqrt;
DVE RoPE muls/adds, aT casts, recip, stats, LN math+normalize;
Pool (gpsimd) RoPE sin-mul, V pack copies, attn normalize.
"""
import sys
import types
import os
import numpy as np
from contextlib import ExitStack

for _p in ("/opt/trn_rl_repo",):
    if _p not in sys.path:
        sys.path.append(_p)

# NTFF profile hook shim: lets BASS_TRACE=1 work in images whose antenv
# lacks axon_hooks (bass_utils imports it when tracing under axon).
if "antenv.axon_hooks" not in sys.modules:
    _hooks = types.ModuleType("antenv.axon_hooks")
    _HOOK = [None]
    _hooks.set_axon_ntff_profile_hook = lambda h: _HOOK.__setitem__(0, h)
    _hooks.get_axon_ntff_profile_hook = lambda: _HOOK[0]
    sys.modules["antenv.axon_hooks"] = _hooks
    try:
        from trn_agent_boot.trn_boot import _ntff_profile_via_ctypes

        _HOOK[0] = _ntff_profile_via_ctypes("/opt/axon/libaxon_pjrt.so")
    except Exception:
        pass

import concourse.bass as bass  # noqa: E402
import concourse.bacc as bacc  # noqa: E402
import concourse.mybir as mybir  # noqa: E402
import concourse.tile as tile  # noqa: E402
from concourse import bass_utils  # noqa: E402

F32 = mybir.dt.float32
BF16 = mybir.dt.bfloat16
FP8 = mybir.dt.float8e4
NP_BF16 = np.dtype(mybir.dt.np(BF16))
AF = mybir.ActivationFunctionType
ALU = mybir.AluOpType
AX = mybir.AxisListType

B, S, D, H, DH = 2, 2048, 1024, 16, 64
NC = 8
R = B * S             # 4096 rows (positions across both batches)
DC = D // 128         # 8 contraction chunks
NSEG = 8              # projection segments of 512 positions
SEGW = R // NSEG      # 512
KT = 32               # global 128-key tiles (16 per batch)
NQB = 4               # 512-wide q blocks per batch
LN_EPS = 1e-5
ROPE_BASE = 10000.0


def _build(flags):
    has_bqk, has_bv, has_gb = flags
    nc = bacc.Bacc("TRN2", target_bir_lowering=False, debug=False,
                   num_devices=NC)

    xqT = nc.dram_tensor("xqT", [NSEG * 128, DC * SEGW], BF16,
                         kind="ExternalInput")
    xvT = nc.dram_tensor("xvT", [NSEG * 128, DC * SEGW], BF16,
                         kind="ExternalInput")
    wq_d = nc.dram_tensor("wq", [D, 128], BF16, kind="ExternalInput")
    wk_d = nc.dram_tensor("wk", [D, 128], BF16, kind="ExternalInput")
    wv_d = nc.dram_tensor("wv", [D, 128], BF16, kind="ExternalInput")
    perm_d = nc.dram_tensor("perm", [128, 128], BF16, kind="ExternalInput")
    ident_d = nc.dram_tensor("ident", [128, 128], BF16, kind="ExternalInput")
    cos_d = nc.dram_tensor("cos", [128, R], F32, kind="ExternalInput")
    sin_d = nc.dram_tensor("sin", [128, R], F32, kind="ExternalInput")
    if has_bqk:
        cq_d = nc.dram_tensor("cq", [128, R], F32, kind="ExternalInput")
        ck_d = nc.dram_tensor("ck", [128, R], F32, kind="ExternalInput")
    if has_bv:
        bv_d = nc.dram_tensor("bv", [128, 128], F32, kind="ExternalInput")
    if has_gb:
        gam_d = nc.dram_tensor("gamma", [128, 128], F32, kind="ExternalInput")
        bet_d = nc.dram_tensor("beta", [128, 128], F32, kind="ExternalInput")
    out_d = nc.dram_tensor("out", [R, 128], F32, kind="ExternalOutput")

    es = ExitStack()
    with es:
        tc = es.enter_context(tile.TileContext(nc))
        dram = es.enter_context(
            tc.tile_pool(name="dram", bufs=1, space="DRAM"))
        constp = es.enter_context(tc.tile_pool(name="const", bufs=1))
        qkp = es.enter_context(tc.tile_pool(name="qkp", bufs=1))
        vfp = es.enter_context(tc.tile_pool(name="vfp", bufs=1))
        ptp = es.enter_context(tc.tile_pool(name="ptp", bufs=2))
        attnp = es.enter_context(tc.tile_pool(name="attnp", bufs=1))
        statp = es.enter_context(tc.tile_pool(name="statp", bufs=1))
        lnp = es.enter_context(tc.tile_pool(name="lnp", bufs=2))
        outp = es.enter_context(tc.tile_pool(name="outp", bufs=4))

        # raw dram tensors (not pool tiles): the LN-tail critical section
        # pre-waits on every open pool tile's last writer -- for a pool-tile
        # st_g that would chain it to AllGather#1's completion and serialize
        # the collective with the whole LN tail.
        st_b = [nc.dram_tensor(f"stb{b}", [128, 32], F32, kind="Internal")
                for b in range(B)]
        st_g = [nc.dram_tensor(f"stg{b}", [NC * 128, 32], F32,
                               kind="Internal", addr_space="Shared")
                for b in range(B)]

        cos_sb = constp.tile([128, R], F32, tag="cos")
        sin_sb = constp.tile([128, R], F32, tag="sin")
        perm_sb = constp.tile([128, 128], BF16, tag="perm")
        ident_sb = constp.tile([128, 128], BF16, tag="ident")
        eps_sb = constp.tile([128, 1], F32, tag="eps")
        nc.vector.memset(eps_sb[:], LN_EPS)

        cq_sb = ck_sb = bv_sb = gam_sb = bet_sb = None
        if has_bqk:
            cq_sb = constp.tile([128, R], F32, tag="cq")
            ck_sb = constp.tile([128, R], F32, tag="ck")
            for hf in range(2):
                sl = slice(hf * 2048, (hf + 1) * 2048)
                nc.sync.dma_start(cq_sb[:, sl], cq_d[:, sl])
                nc.sync.dma_start(ck_sb[:, sl], ck_d[:, sl])
        if has_bv:
            bv_sb = constp.tile([128, 128], F32, tag="bvs")
            nc.sync.dma_start(bv_sb[:], bv_d[:])
        if has_gb:
            gam_sb = constp.tile([128, 128], F32, tag="gam")
            nc.sync.dma_start(gam_sb[:], gam_d[:])
            bet_sb = constp.tile([128, 128], F32, tag="bet")
            nc.sync.dma_start(bet_sb[:], bet_d[:])

        # [dh-of-pair (h0: 0-63, h1: 64-127), b*2048 + s]
        q_sb = qkp.tile([128, R], BF16, tag="q")
        k_sb = qkp.tile([128, R], BF16, tag="k")
        q1_sb = qkp.tile([64, R], BF16, tag="q1")   # partitions 64:128 -> 0:64
        k1_sb = qkp.tile([64, R], BF16, tag="k1")
        KFP8 = os.environ.get("KFP8", "0") == "1"
        PDT = FP8 if KFP8 else BF16
        # [key-in-tile, kt*130 + hl*65 + (dh | ones)]
        v_sb = vfp.tile([128, KT * 130], PDT, tag="v")
        # [q-in-tile, tt*128 + hl*64 + dh] for row-tile tt
        attn_sb = attnp.tile([128, 32 * 128], F32, tag="attn")
        # [row, tt*2 + (sum|sumsq)]
        stats_sb = statp.tile([128, 64], F32, tag="stats")

        # ---------------- projections ----------------
        pes = ExitStack()
        with pes:
            xqp = pes.enter_context(tc.tile_pool(name="xqp", bufs=3))
            xvp = pes.enter_context(tc.tile_pool(name="xvp", bufs=2))
            wp = pes.enter_context(tc.tile_pool(name="wp", bufs=1))
            usbp = pes.enter_context(tc.tile_pool(name="usbp", bufs=4))
            stage = pes.enter_context(tc.tile_pool(name="stage", bufs=6))
            pjp = pes.enter_context(
                tc.tile_pool(name="pjp", bufs=6, space="PSUM"))
            pvp = pes.enter_context(
                tc.tile_pool(name="pvp", bufs=2, space="PSUM"))

            def load_w(t_dram, tg):
                # one 3-D DMA: [dc, p, col] -> [p, dc*128 + col]
                w_sb = wp.tile([128, DC * 128], BF16, tag=tg)
                nc.sync.dma_start(
                    w_sb[:].rearrange("p (dc c) -> p dc c", c=128),
                    t_dram[:].rearrange("(dc p) c -> p dc c", p=128))
                return w_sb

            def load_xseg(pool, src, seg, tg, eng):
                # host pre-tiled: one plain 2-D DMA per segment
                t = pool.tile([128, DC * SEGW], BF16, tag=tg)
                eng.dma_start(t[:], src[seg * 128:(seg + 1) * 128, :])
                return t

            wq_sb = load_w(wq_d, "wq")
            nc.sync.dma_start(perm_sb[:], perm_d[:])
            xsegs = [load_xseg(xqp, xqT, s_, "xq", nc.sync)
                     for s_ in range(3)]
            wk_sb = load_w(wk_d, "wk")
            wv_sb = load_w(wv_d, "wv")
            nc.gpsimd.dma_start(ident_sb[:], ident_d[:])
            for hf in range(2):
                slh = slice(hf * 2048, (hf + 1) * 2048)
                nc.scalar.dma_start(cos_sb[:, slh], cos_d[:, slh])
                nc.scalar.dma_start(sin_sb[:, slh], sin_d[:, slh])

            # ones in v_sb (cols kt*130 + hl*65 + 64)
            v3 = v_sb[:].rearrange("p (x e) -> p x e", e=65)
            nc.gpsimd.memset(v3[:, :, 64:65], 1.0)

            def proj_u(w_sb, xseg, seg, nm):
                ps_u = pjp.tile([128, SEGW], F32, tag="pj",
                                name=f"psu_{nm}_{seg}")
                for dc in range(DC):
                    nc.tensor.matmul(
                        ps_u[:],
                        w_sb[:, dc * 128:(dc + 1) * 128],
                        xseg[:, dc * SEGW:(dc + 1) * SEGW],
                        start=(dc == 0), stop=(dc == DC - 1))
                return ps_u

            def rope(ps_u, seg, c_sb, dst, dst1, nm):
                sl = slice(seg * SEGW, (seg + 1) * SEGW)
                u_sb = usbp.tile([128, SEGW], BF16, tag="usb",
                                 name=f"usb_{nm}_{seg}")
                nc.scalar.copy(u_sb[:], ps_u[:])                     # Act
                ps_u2 = pjp.tile([128, SEGW], F32, tag="pj",
                                 name=f"psu2_{nm}_{seg}")
                nc.tensor.matmul(ps_u2[:], perm_sb[:], u_sb[:],
                                 start=True, stop=True)
                t1 = stage.tile([128, SEGW], F32, tag="st",
                                name=f"t1_{nm}_{seg}")
                nc.vector.tensor_mul(t1[:], ps_u[:], cos_sb[:, sl])  # DVE
                t2 = stage.tile([128, SEGW], F32, tag="st",
                                name=f"t2_{nm}_{seg}")
                nc.vector.tensor_mul(t2[:], ps_u2[:], sin_sb[:, sl])  # DVE
                if c_sb is None:
                    nc.vector.tensor_add(dst[:, sl], t1[:], t2[:])   # DVE
                else:
                    t3 = stage.tile([128, SEGW], F32, tag="st",
                                    name=f"t3_{nm}_{seg}")
                    nc.vector.tensor_add(t3[:], t1[:], t2[:])
                    nc.vector.tensor_add(dst[:, sl], t3[:], c_sb[:, sl])
                nc.scalar.dma_start(dst1[:, sl], dst[64:128, sl])

            def v_proj(xvseg, seg):
                for j in range(4):
                    kt = seg * 4 + j
                    ps_v = pvp.tile([128, 128], F32, tag="pv",
                                    name=f"psv_{kt}")
                    for dc in range(DC):
                        nc.tensor.matmul(
                            ps_v[:],
                            xvseg[:, dc * SEGW + j * 128:
                                  dc * SEGW + (j + 1) * 128],
                            wv_sb[:, dc * 128:(dc + 1) * 128],
                            start=(dc == 0), stop=(dc == DC - 1))
                    dstv = v_sb[:, kt * 130:(kt + 1) * 130].rearrange(
                        "p (h e) -> p h e", e=65)[:, :, 0:64]
                    srcv = ps_v[:].rearrange("p (h d) -> p h d", d=64)
                    if has_bv:
                        bvv = bv_sb[:].rearrange("p (h d) -> p h d", d=64)
                        nc.vector.tensor_add(dstv, srcv, bvv)
                    else:
                        nc.vector.tensor_copy(dstv, srcv)            # DVE

            for seg in range(NSEG):
                xseg = xsegs[seg] if seg < 3 else load_xseg(
                    xqp, xqT, seg, "xq", nc.sync)
                ps_q = proj_u(wq_sb, xseg, seg, "q")
                ps_k = proj_u(wk_sb, xseg, seg, "k")
                rope(ps_q, seg, cq_sb, q_sb, q1_sb, "q")
                rope(ps_k, seg, ck_sb, k_sb, k1_sb, "k")
            for seg in range(NSEG):
                xvseg = load_xseg(xvp, xvT, seg, "xv", nc.gpsimd)
                v_proj(xvseg, seg)

        # Full cross-engine fence: attention-phase PSUM tiles reuse the
        # projection pools' banks, and a matmul start=True zeroes its whole
        # 2KB PSUM bank -- without the fence that clobbers proj psum tiles
        # that still have pending readers (zero-region isn't visible to the
        # tile dependency tracker).
        with tc.tile_critical(name="proj_done"):
            pass

        # ---------------- attention ----------------
        aes = ExitStack()
        with aes:
            scp = aes.enter_context(
                tc.tile_pool(name="scp", bufs=2, space="PSUM"))
            avp = aes.enter_context(
                tc.tile_pool(name="avp", bufs=1, space="PSUM"))
            trp = aes.enter_context(
                tc.tile_pool(name="trp", bufs=1, space="PSUM"))
            atsb = aes.enter_context(tc.tile_pool(name="atsb", bufs=2))
            recp = aes.enter_context(tc.tile_pool(name="recp", bufs=2))
            sqp = aes.enter_context(tc.tile_pool(name="sqp", bufs=2))

            # unit = (b, qb, hl); per batch, qb-major with hl inner so a
            # (qb, both heads) pair completes 4 row-tiles of attn_sb.
            units = [(b, qb, hl)
                     for b in range(B) for qb in range(NQB) for hl in range(2)]

            def stage1(u):
                """scores + exp -> pt tile (returned)."""
                b, qb, hl = u
                ksrc = k_sb if hl == 0 else k1_sb
                qsrc = q_sb if hl == 0 else q1_sb
                qsl = slice(b * S + qb * 512, b * S + (qb + 1) * 512)
                pt = ptp.tile([128, 16 * 512], PDT, tag="pt",
                              name=f"pt_{b}_{qb}_{hl}")
                # kt chunks of (3,3,3,3,2,2): bigger ACTIVATE instrs
                # amortize the ~480ns fixed overhead on the exp stream.
                kt0 = 0
                for st, w in enumerate((3, 3, 3, 3, 2, 2)):
                    ps_s = scp.tile([128, 1536], F32, tag="sc",
                                    name=f"sc_{b}_{qb}_{hl}_{st}")
                    for j in range(w):
                        ktb = kt0 + j
                        nc.tensor.matmul(
                            ps_s[:, j * 512:(j + 1) * 512],
                            ksrc[0:64, b * S + ktb * 128:
                                 b * S + (ktb + 1) * 128],
                            qsrc[0:64, qsl],
                            start=True, stop=True)
                    nc.scalar.activation(
                        pt[:, kt0 * 512:(kt0 + w) * 512],
                        ps_s[:, 0:w * 512], AF.Exp, scale=0.125)
                    kt0 += w
                return pt

            def stage2(u, pt):
                """AV + transpose + normalize -> attn_sb columns."""
                b, qb, hl = u
                aT = avp.tile([65, 512], F32, tag="av",
                              name=f"aT_{b}_{qb}_{hl}")
                if KFP8:
                    # fp8 DoubleRow: two k-tiles per matmul at 2x rate
                    vv = v_sb[:].rearrange("p (k x) -> p k x", x=130)
                    pv = pt[:].rearrange("p (k n) -> p k n", n=512)
                    for pr in range(8):
                        kt = b * 16 + pr * 2
                        nc.tensor.matmul(
                            aT[:],
                            vv[:, kt:kt + 2, hl * 65:hl * 65 + 65],
                            pv[:, pr * 2:pr * 2 + 2, :],
                            start=(pr == 0), stop=(pr == 7),
                            perf_mode=mybir.MatmulPerfMode.DoubleRow)
                else:
                    for ktb in range(16):
                        kt = b * 16 + ktb
                        nc.tensor.matmul(
                            aT[:],
                            v_sb[:, kt * 130 + hl * 65:
                                 kt * 130 + hl * 65 + 65],
                            pt[:, ktb * 512:(ktb + 1) * 512],
                            start=(ktb == 0), stop=(ktb == 15))
                aT_sb = atsb.tile([65, 512], BF16, tag="ats",
                                  name=f"ats_{b}_{qb}_{hl}")
                nc.vector.tensor_copy(aT_sb[:], aT[:])               # DVE
                tr = trp.tile([128, 264], BF16, tag="tr",
                              name=f"tr_{b}_{qb}_{hl}")
                for t in range(4):
                    nc.tensor.transpose(
                        tr[:, t * 66: t * 66 + 65],
                        aT_sb[:, t * 128:(t + 1) * 128],
                        ident_sb[0:65, 0:65])
                tr_sb = atsb.tile([128, 260], BF16, tag="trs",
                                  name=f"trs_{b}_{qb}_{hl}")
                nc.vector.tensor_copy(                               # DVE 2x
                    tr_sb[:].rearrange("p (t e) -> p t e", e=65),
                    tr[:].rearrange("p (t e) -> p t e", e=66)[:, :, 0:65])
                rec = recp.tile([128, 4], F32, tag="rec",
                                name=f"rec_{b}_{qb}_{hl}")
                nc.vector.reciprocal(rec[:], tr_sb[:, 64::65])       # DVE
                for t in range(4):
                    tt = b * 16 + qb * 4 + t
                    nc.vector.tensor_scalar(                         # DVE
                        attn_sb[:, tt * 128 + hl * 64:
                                tt * 128 + hl * 64 + 64],
                        tr_sb[:, t * 65: t * 65 + 64],
                        rec[:, t: t + 1], None, ALU.mult)

            def stats_qb(b, qb):
                for t in range(4):
                    tt = b * 16 + qb * 4 + t
                    at = attn_sb[:, tt * 128:(tt + 1) * 128]
                    nc.vector.reduce_sum(
                        stats_sb[:, 2 * tt: 2 * tt + 1], at, axis=AX.X)
                    sq = sqp.tile([128, 128], F32, tag="sq",
                                  name=f"sq_{tt}")
                    nc.vector.tensor_mul(sq[:], at, at)
                    nc.vector.reduce_sum(
                        stats_sb[:, 2 * tt + 1: 2 * tt + 2], sq[:],
                        axis=AX.X)

            def stats_flush(b):
                nc.sync.dma_start(st_b[b][:],
                                  stats_sb[:, b * 32:(b + 1) * 32])
                nc.gpsimd.collective_compute(
                    "AllGather", ALU.bypass,
                    ins=[st_b[b][:].opt()], outs=[st_g[b][:].opt()],
                    replica_groups=[list(range(NC))])

            def ln_half(b, lnp, outp):
                tot8 = lnp.tile([128, 8 * 32], F32, tag="tot8",
                                name=f"tot8{b}")
                nc.sync.dma_start(
                    tot8[:].rearrange("p (c w) -> p c w", w=32),
                    st_g[b][:].rearrange("(c p) w -> p c w", p=128))
                tot = lnp.tile([128, 32], F32, tag="tot", name=f"tot{b}")
                nc.vector.tensor_add(tot[:], tot8[:, 0:32], tot8[:, 32:64])
                for c in range(2, NC):
                    nc.vector.tensor_add(tot[:], tot[:],
                                         tot8[:, c * 32:(c + 1) * 32])
                nmu = lnp.tile([128, 16], F32, tag="nmu", name=f"nmu{b}")
                nc.vector.tensor_scalar_mul(nmu[:], tot[:, 0::2], -1.0 / D)
                ex2 = lnp.tile([128, 16], F32, tag="ex2", name=f"ex2{b}")
                nc.vector.tensor_scalar_mul(ex2[:], tot[:, 1::2], 1.0 / D)
                var = lnp.tile([128, 16], F32, tag="var", name=f"var{b}")
                nc.vector.tensor_tensor(var[:], nmu[:], nmu[:], ALU.mult)
                nc.vector.tensor_tensor(var[:], ex2[:], var[:], ALU.subtract)
                std = lnp.tile([128, 16], F32, tag="std", name=f"std{b}")
                nc.scalar.activation(std[:], var[:], AF.Sqrt,
                                     bias=eps_sb[:])                 # Act
                rstd = lnp.tile([128, 16], F32, tag="rstd", name=f"rstd{b}")
                nc.vector.reciprocal(rstd[:], std[:])
                mrs = lnp.tile([128, 16], F32, tag="mrs", name=f"mrs{b}")
                nc.vector.tensor_tensor(mrs[:], nmu[:], rstd[:], ALU.mult)
                o_sb = outp.tile([128, 16 * 128], F32, tag="o",
                                 name=f"o_{b}")
                for t in range(16):
                    tt = b * 16 + t
                    osl = o_sb[:, t * 128:(t + 1) * 128]
                    if t % 2 == 0:
                        nc.vector.tensor_scalar(                     # DVE
                            osl, attn_sb[:, tt * 128:(tt + 1) * 128],
                            rstd[:, t: t + 1], mrs[:, t: t + 1],
                            ALU.mult, ALU.add)
                    else:
                        nc.scalar.activation(                        # Act
                            osl, attn_sb[:, tt * 128:(tt + 1) * 128],
                            AF.Identity, bias=mrs[:, t: t + 1],
                            scale=rstd[:, t: t + 1])
                    if has_gb:
                        nc.vector.tensor_tensor(
                            osl, osl, gam_sb[:], ALU.mult)
                        nc.vector.tensor_tensor(
                            osl, osl, bet_sb[:], ALU.add)
                # one 3-D DMA: [p, t, col] -> out rows (b*16+t)*128+p
                nc.sync.dma_start(
                    out_d[b * 2048:(b + 1) * 2048, :].rearrange(
                        "(t p) c -> p t c", p=128),
                    o_sb[:].rearrange("p (t c) -> p t c", c=128))

            # KSTAGE: 2 = attention only; 25 = +stats; 26 = +collective
            # +readback; 30 = full (LN math + normalize)
            KSTAGE = int(os.environ.get("KSTAGE", "30"))
            pend = None
            for i, u in enumerate(units):
                pt = stage1(u)
                if pend is not None:
                    stage2(*pend)
                    pb, pqb, phl = pend[0]
                    if KSTAGE >= 25 and phl == 1:
                        stats_qb(pb, pqb)
                        if KSTAGE >= 26 and pqb == NQB - 1:
                            stats_flush(pb)
                pend = (u, pt)
            stage2(*pend)
            if KSTAGE >= 25:
                stats_qb(B - 1, NQB - 1)
            if KSTAGE >= 26:
                stats_flush(B - 1)
            if KSTAGE >= 30:
                # The LN tail must not be scheduled into the attention
                # stream: its first ops wait on the stats AllGathers, and
                # hoisting them blocks the whole in-order DVE queue (observed
                # 50us engine stalls).  tile_critical + a nested TileContext
                # (the supported pattern) pins them after the attention
                # drain, with their own pools.
                with tc.tile_critical(name="ln_tail"):
                    with tile.TileContext(nc) as tc2:
                        with tc2.tile_pool(name="lnp2", bufs=2) as lnp2, \
                                tc2.tile_pool(name="outp2", bufs=2) as outp2:
                            ln_half(0, lnp2, outp2)
                            ln_half(1, lnp2, outp2)
            else:
                if KSTAGE >= 26:
                    # read back the gathered stats (sums unused downstream)
                    for b in range(B):
                        tot8 = lnp.tile([128, 8 * 32], F32, tag="tot8",
                                        name=f"tot8d{b}")
                        for c in range(NC):
                            nc.sync.dma_start(
                                tot8[:, c * 32:(c + 1) * 32],
                                st_g[b][c * 128:(c + 1) * 128, :])
                        tot = lnp.tile([128, 32], F32, tag="tot",
                                       name=f"totd{b}")
                        nc.vector.tensor_add(tot[:], tot8[:, 0:32],
                                             tot8[:, 32:64])
                # debug: dump raw attn (no LN) so outputs are produced
                for tt in range(32):
                    o_sb = outp.tile([128, 128], F32, tag="o",
                                     name=f"od_{tt}")
                    nc.vector.tensor_copy(
                        o_sb[:], attn_sb[:, tt * 128:(tt + 1) * 128])
                    nc.sync.dma_start(out_d[tt * 128:(tt + 1) * 128, :],
                                      o_sb[:])

    nc.compile()
    return nc


_CACHE: dict = {}
LAST_EXEC_NS = None


def _rope_tables():
    half = DH // 2
    inv_freq = 1.0 / (ROPE_BASE ** (np.arange(half, dtype=np.float32) / half))
    t = np.arange(S, dtype=np.float32)
    freqs = t[:, None] * inv_freq[None, :]
    emb = np.concatenate([freqs, freqs], axis=-1)          # [S, DH]
    return np.cos(emb).astype(np.float32), np.sin(emb).astype(np.float32)


def prep_flags(inputs):
    b_qk = np.asarray(inputs["b_qk"], dtype=np.float32)
    b_v = np.asarray(inputs["b_v"], dtype=np.float32)
    gamma = np.asarray(inputs["ln_gamma"], dtype=np.float32)
    beta = np.asarray(inputs["ln_beta"], dtype=np.float32)
    return (bool(np.any(b_qk)), bool(np.any(b_v)),
            bool(np.any(gamma != 1.0) or np.any(beta != 0.0)))


def _perm_mat():
    Pm = np.zeros((128, 128), np.float32)
    for i in range(64):
        Pm[2 * i + 1, 2 * i] = -1.0
        Pm[2 * i, 2 * i + 1] = 1.0
    return Pm


def _prep_in_maps(inputs, flags):
    x_qk = np.asarray(inputs["x_qk"], dtype=np.float32)
    x_v = np.asarray(inputs["x_v"], dtype=np.float32)
    W_qk = np.asarray(inputs["W_qk"], dtype=np.float32)
    b_qk = np.asarray(inputs["b_qk"], dtype=np.float32)
    W_v = np.asarray(inputs["W_v"], dtype=np.float32)
    b_v = np.asarray(inputs["b_v"], dtype=np.float32)
    gamma = np.asarray(inputs["ln_gamma"], dtype=np.float32)
    beta = np.asarray(inputs["ln_beta"], dtype=np.float32)

    Pm = _perm_mat()
    Pm64 = Pm[:DH, :DH]
    cos_all, sin_all = _rope_tables()          # [S, 64]
    cos_in = np.ascontiguousarray(np.tile(cos_all.T, (2, 2)))  # [128, 4096]
    sin_in = np.ascontiguousarray(np.tile(sin_all.T, (2, 2)))

    Wq = W_qk[:, :D]
    Wk = W_qk[:, D:]
    bq = b_qk[:D].reshape(H, DH)
    bk = b_qk[D:].reshape(H, DH)
    bq2 = bq @ Pm64
    bk2 = bk @ Pm64

    def seg_tile(x):
        # [seg*128 + p, dc*SEGW + c] = x[seg*SEGW + c, dc*128 + p]
        return np.ascontiguousarray(
            x.reshape(NSEG, SEGW, DC, 128).transpose(0, 3, 2, 1)
            .reshape(NSEG * 128, DC * SEGW).astype(NP_BF16))

    xqT_np = seg_tile(x_qk.reshape(R, D))
    xvT_np = seg_tile(x_v.reshape(R, D))
    perm_np = np.ascontiguousarray(Pm.astype(NP_BF16))
    ident_np = np.ascontiguousarray(np.eye(128, dtype=NP_BF16))

    in_maps = []
    for c in range(NC):
        cols = slice(c * 128, (c + 1) * 128)
        m = {
            "xqT": xqT_np, "xvT": xvT_np,
            "wq": np.ascontiguousarray(Wq[:, cols].astype(NP_BF16)),
            "wk": np.ascontiguousarray(Wk[:, cols].astype(NP_BF16)),
            "wv": np.ascontiguousarray(W_v[:, cols].astype(NP_BF16)),
            "perm": perm_np, "ident": ident_np,
            "cos": cos_in, "sin": sin_in,
        }
        if flags[0]:
            # additive post-RoPE bias tables for this head pair
            def fold(bh, bh2):
                rows = [bh[2 * c + hl][:, None] * cos_all.T
                        + bh2[2 * c + hl][:, None] * sin_all.T
                        for hl in range(2)]          # each [64, S]
                return np.ascontiguousarray(
                    np.tile(np.vstack(rows), (1, 2)).astype(np.float32))
            m["cq"] = fold(bq, bq2)
            m["ck"] = fold(bk, bk2)
        if flags[1]:
            m["bv"] = np.ascontiguousarray(np.broadcast_to(
                b_v[c * 128:(c + 1) * 128], (128, 128)).astype(np.float32))
        if flags[2]:
            m["gamma"] = np.ascontiguousarray(np.broadcast_to(
                gamma[c * 128:(c + 1) * 128], (128, 128)).astype(np.float32))
            m["beta"] = np.ascontiguousarray(np.broadcast_to(
                beta[c * 128:(c + 1) * 128], (128, 128)).astype(np.float32))
        in_maps.append(m)
    return in_maps


def kernel(**inputs):
    flags = prep_flags(inputs)
    if flags not in _CACHE:
        _CACHE[flags] = _build(flags)
    nc = _CACHE[flags]
    in_maps = _prep_in_maps(inputs, flags)
    res = bass_utils.run_bass_kernel_spmd(
        nc, in_maps, core_ids=list(range(NC)))
    global LAST_EXEC_NS
    LAST_EXEC_NS = res.exec_time_ns
    out = np.empty((R, D), np.float32)
    for c in range(NC):
        out[:, c * 128:(c + 1) * 128] = np.asarray(
            res.results[c]["out"], dtype=np.float32)
    return out.reshape(B, S, D)
